# revision 4
# baseline (speedup 1.0000x reference)
"""Trainium2 Bass kernel for nn_Attention_56736517980223.

Full-input contract: kernel(**inputs) takes the unsharded inputs and returns
the full [2, 2048, 2048] attention output. Internally: tensor-parallel over
heads across 8 NeuronCores (1 KV head + 4 Q heads per core); each core
computes its heads' attention and a partial x@wo contribution; the host sums
the 8 partials.

Device-side dataflow per core (all matmuls in float32r = TF32):
  - xT (host-pretransposed [dim, tok]) streams as the moving operand of the
    QKV projections; weight columns are even/odd-permuted so RoPE acts on
    contiguous 32-partition blocks.
  - scores are computed transposed (S^T[k, q] = kT.T-chunk @ qT) so softmax
    needs no reductions: exp(scale*s) runs on ScalarE straight out of PSUM,
    causal masking is a staircase zero-fill (gpsimd affine_select), and the
    row sums fall out of the PV matmul via a ones-column appended to V.
  - PV output [d+1, q] is normalized with reciprocal + partition_broadcast
    and written into attnT, which feeds the wo matmul as the stationary
    operand. Final PSUM evacuation is split across VectorE and ScalarE.
"""

import numpy as np

DIM = 2048
N_HEADS = 32
N_KV_HEADS = 8
HEAD_DIM = 64
BATCH = 2
SEQ = 2048
TOK = BATCH * SEQ  # 4096
N_CORES = 8
HPC = N_HEADS // N_KV_HEADS  # 4 q heads per core
CHUNK = 512  # token chunk (projection streaming / q block)
KC = 128     # key chunk (scores partition dim)
NQB = SEQ // CHUNK   # 4 q blocks per batch
NKC = SEQ // KC      # 16 key chunks per batch
SCALE = 1.0 / np.sqrt(HEAD_DIM)

_CACHE = {}
LAST_RESULT = None


def _build(tile_types, generic):
    """Build the SPMD Bass program.

    tile_types[qc][kc] in {'full', 'diag', 'skip', 'gen'} (batch-local,
    shared across batches and heads). 'diag' uses the causal affine_select;
    'gen' adds a DMA'd mask tile (only in generic mode).
    """
    from contextlib import ExitStack
    import concourse.bass as bass
    import concourse.tile as tile
    from concourse import bacc, mybir

    F32 = mybir.dt.float32
    F32R = mybir.dt.float32r
    AF = mybir.ActivationFunctionType
    ALU = mybir.AluOpType

    nc = bacc.Bacc("TRN2", target_bir_lowering=False, debug=False,
                   num_devices=N_CORES)

    xt = nc.dram_tensor("xt", [DIM, TOK], F32R, kind="ExternalInput").ap()
    wq = nc.dram_tensor("wq", [DIM, 2 * KC], F32R, kind="ExternalInput").ap()
    wkv = nc.dram_tensor("wkv", [DIM, KC], F32R, kind="ExternalInput").ap()
    wo1 = nc.dram_tensor("wo1", [KC, DIM], F32R, kind="ExternalInput").ap()
    wo2 = nc.dram_tensor("wo2", [KC, DIM], F32R, kind="ExternalInput").ap()
    cos_q = nc.dram_tensor("cos_q", [KC, TOK], F32, kind="ExternalInput").ap()
    sin_q = nc.dram_tensor("sin_q", [KC, TOK], F32, kind="ExternalInput").ap()
    if generic:
        maskt = nc.dram_tensor("maskt", [SEQ, SEQ], F32,
                               kind="ExternalInput").ap()
    out = nc.dram_tensor("out", [TOK, DIM], F32, kind="ExternalOutput").ap()

    NCH = TOK // CHUNK  # 8 token chunks
    NKT = DIM // KC     # 16 contraction tiles for projections

    with tile.TileContext(nc) as tc, ExitStack() as ctx:
        persist = ctx.enter_context(tc.tile_pool(name="persist", bufs=1))
        qt1 = persist.tile([KC, TOK], F32R)   # heads 0,1 (rows 0:64 / 64:128)
        qt2 = persist.tile([KC, TOK], F32R)   # heads 2,3
        kt = persist.tile([KC, TOK], F32R)    # rows 0:64 = kT, 64:128 = dup
        ident = persist.tile([64, 64], F32R)
        nc.gpsimd.memset(ident[:].bitcast(F32), 0.0)
        nc.gpsimd.affine_select(
            out=ident[:], in_=ident[:], compare_op=ALU.not_equal,
            fill=1.0, base=0, channel_multiplier=1, pattern=[[-1, 64]])

        vpool = ctx.enter_context(tc.tile_pool(name="vpool", bufs=1))
        vt = vpool.tile([64, TOK], F32R)
        v_all = vpool.tile([KC, (TOK // KC) * 65], F32R)  # 32 [128,65] blocks

        # ---------------- projection + RoPE + V transpose ----------------
        with tc.tile_pool(name="proj", bufs=1) as proj, \
             tc.tile_pool(name="projs", bufs=2) as projs, \
             tc.tile_pool(name="ropet", bufs=2) as ropet, \
             tc.tile_pool(name="pps", bufs=2, space="PSUM") as pps:
            wq_sb = proj.tile([KC, NKT * 2 * KC], F32R)
            wkv_sb = proj.tile([KC, NKT * KC], F32R)
            nc.sync.dma_start(
                wq_sb[:].rearrange("p (t m) -> p t m", t=NKT),
                wq.rearrange("(t p) m -> p t m", p=KC))
            nc.sync.dma_start(
                wkv_sb[:].rearrange("p (t m) -> p t m", t=NKT),
                wkv.rearrange("(t p) m -> p t m", p=KC))

            for ch in range(NCH):
                tsl = slice(ch * CHUNK, (ch + 1) * CHUNK)
                xt_sb = []
                for kti in range(NKT):
                    xtile = projs.tile([KC, CHUNK], F32R, name=f"xtile{kti}",
                                       tag=f"xtile{kti}")
                    nc.sync.dma_start(
                        xtile[:], xt[kti * KC:(kti + 1) * KC, tsl])
                    xt_sb.append(xtile)
                cos_sb = projs.tile([KC, CHUNK], F32)
                sin_sb = projs.tile([KC, CHUNK], F32)
                nc.sync.dma_start(cos_sb[:], cos_q[:, tsl])
                nc.sync.dma_start(sin_sb[:], sin_q[:, tsl])

                a_ps = pps.tile([KC, CHUNK], F32, tag="a_ps")
                b_ps = pps.tile([KC, CHUNK], F32, tag="b_ps")
                kv_ps = pps.tile([KC, CHUNK], F32, tag="kv_ps")
                for kti in range(NKT):
                    st = (kti == 0)
                    sp = (kti == NKT - 1)
                    nc.tensor.matmul(
                        a_ps[:], wq_sb[:, kti * 256:kti * 256 + 128],
                        xt_sb[kti][:], start=st, stop=sp)
                    nc.tensor.matmul(
                        b_ps[:], wq_sb[:, kti * 256 + 128:kti * 256 + 256],
                        xt_sb[kti][:], start=st, stop=sp)
                    nc.tensor.matmul(
                        kv_ps[:], wkv_sb[:, kti * 128:kti * 128 + 128],
                        xt_sb[kti][:], start=st, stop=sp)

                # Q RoPE: A' = A*c - B*s ; B' = A*s + B*c  (to temps, then
                # DMA-rearrange into per-head-contiguous qt1/qt2)
                t1 = ropet.tile([KC, CHUNK], F32, tag="t1")
                t2 = ropet.tile([KC, CHUNK], F32, tag="t2")
                qa = ropet.tile([KC, CHUNK], F32R, tag="qa")
                qb = ropet.tile([KC, CHUNK], F32R, tag="qb")
                nc.vector.tensor_mul(t1[:], a_ps[:], cos_sb[:])
                nc.vector.tensor_mul(t2[:], b_ps[:], sin_sb[:])
                nc.vector.tensor_sub(qa[:], t1[:], t2[:])
                t3 = ropet.tile([KC, CHUNK], F32, tag="t1")
                t4 = ropet.tile([KC, CHUNK], F32, tag="t2")
                nc.vector.tensor_mul(t3[:], a_ps[:], sin_sb[:])
                nc.vector.tensor_mul(t4[:], b_ps[:], cos_sb[:])
                nc.vector.tensor_add(qb[:], t3[:], t4[:])
                for h in range(HPC):
                    dst = qt1 if h < 2 else qt2
                    r0 = 64 * (h % 2)
                    nc.sync.dma_start(dst[r0:r0 + 32, tsl],
                                      qa[32 * h:32 * h + 32, :])
                    nc.sync.dma_start(dst[r0 + 32:r0 + 64, tsl],
                                      qb[32 * h:32 * h + 32, :])

                # K RoPE into kt rows 0:64 (kv_ps rows 0:32=x0, 32:64=x1)
                k1 = ropet.tile([32, CHUNK], F32, tag="k1")
                k2 = ropet.tile([32, CHUNK], F32, tag="k2")
                nc.vector.tensor_mul(k1[:], kv_ps[0:32, :], cos_sb[0:32, :])
                nc.vector.tensor_mul(k2[:], kv_ps[32:64, :], sin_sb[0:32, :])
                nc.vector.tensor_sub(kt[0:32, tsl], k1[:], k2[:])
                k3 = ropet.tile([32, CHUNK], F32, tag="k1")
                k4 = ropet.tile([32, CHUNK], F32, tag="k2")
                nc.vector.tensor_mul(k3[:], kv_ps[0:32, :], sin_sb[0:32, :])
                nc.vector.tensor_mul(k4[:], kv_ps[32:64, :], cos_sb[0:32, :])
                nc.vector.tensor_add(kt[32:64, tsl], k3[:], k4[:])
                # duplicate kT rows for base-64 rhs matmuls
                nc.sync.dma_start(kt[64:128, tsl], kt[0:64, tsl])
                # V: evacuate psum rows 64:128 to vt
                nc.vector.tensor_copy(vt[0:64, tsl], kv_ps[64:128, :])

            # V transpose: vt [64, tok] -> v_all blocks [128, 65] ([V | 1])
            for blk in range(TOK // KC):
                vp = pps.tile([KC, 64], F32R, tag="vp")
                nc.tensor.transpose(
                    vp[:], vt[0:64, blk * KC:(blk + 1) * KC], ident[:])
                nc.vector.tensor_copy(v_all[:, blk * 65:blk * 65 + 64], vp[:])
                nc.gpsimd.memset(
                    v_all[:, blk * 65 + 64:blk * 65 + 65].bitcast(F32), 1.0)

        # ---------------- attention ----------------
        attn = ctx.enter_context(tc.tile_pool(name="attn", bufs=1))
        attnt1 = attn.tile([KC, TOK], F32R)
        attnt2 = attn.tile([KC, TOK], F32R)

        with tc.tile_pool(name="att", bufs=2) as att, \
             tc.tile_pool(name="atps", bufs=1, space="PSUM") as atps:
            for b in range(BATCH):
                for qc in range(NQB):
                    q0 = qc * CHUNK                    # batch-local q offset
                    gq = slice(b * SEQ + q0, b * SEQ + q0 + CHUNK)
                    kcs = [k for k in range(NKC)
                           if tile_types[qc][k] != 'skip']
                    pv = [atps.tile([65, CHUNK], F32, tag=f"pv{h}",
                                    name=f"pv{h}")
                          for h in range(HPC)]
                    for i, kci in enumerate(kcs):
                        k0 = kci * KC
                        gk = slice(b * SEQ + k0, b * SEQ + k0 + KC)
                        ty = tile_types[qc][kci]
                        st = (i == 0)
                        sp = (i == len(kcs) - 1)
                        for pair in range(2):
                            qt = qt1 if pair == 0 else qt2
                            s_ps = atps.tile([KC, 2 * CHUNK], F32,
                                             tag=f"s{pair}", name=f"s{pair}")
                            nc.tensor.matmul(
                                s_ps[:, 0:CHUNK], kt[0:64, gk],
                                qt[0:64, gq], start=True, stop=True)
                            nc.tensor.matmul(
                                s_ps[:, CHUNK:], kt[64:128, gk],
                                qt[64:128, gq], start=True, stop=True)
                            ex = att.tile([KC, 2 * CHUNK], F32R,
                                          tag=f"ex{pair}", name=f"ex{pair}")
                            if ty == 'gen':
                                mt = att.tile([KC, CHUNK], F32, tag="mt",
                                              name="mt", bufs=4)
                                nc.sync.dma_start(
                                    mt[:], maskt[k0:k0 + KC, q0:q0 + CHUNK])
                                for hh in range(2):
                                    csl = slice(hh * CHUNK, (hh + 1) * CHUNK)
                                    tm = att.tile([KC, CHUNK], F32, tag="tm",
                                                  name="tm", bufs=4)
                                    nc.vector.scalar_tensor_tensor(
                                        tm[:], s_ps[:, csl], SCALE, mt[:],
                                        op0=ALU.mult, op1=ALU.add)
                                    nc.scalar.activation(
                                        ex[:, csl], tm[:], AF.Exp)
                            else:
                                nc.scalar.activation(
                                    ex[:], s_ps[:], AF.Exp, scale=SCALE)
                                if ty == 'diag':
                                    for hh in range(2):
                                        csl = slice(hh * CHUNK,
                                                    (hh + 1) * CHUNK)
                                        nc.gpsimd.affine_select(
                                            out=ex[:, csl], in_=ex[:, csl],
                                            compare_op=ALU.is_ge, fill=0.0,
                                            base=q0 - k0,
                                            channel_multiplier=-1,
                                            pattern=[[1, CHUNK]])
                            vblk = (b * SEQ + k0) // KC
                            for hh in range(2):
                                h = 2 * pair + hh
                                csl = slice(hh * CHUNK, (hh + 1) * CHUNK)
                                nc.tensor.matmul(
                                    pv[h][:],
                                    v_all[:, vblk * 65:vblk * 65 + 65],
                                    ex[:, csl], start=st, stop=sp)
                    for h in range(HPC):
                        rec = att.tile([1, CHUNK], F32, tag="rec", name="rec",
                                       bufs=4)
                        bc = att.tile([64, CHUNK], F32, tag="bc", name="bc",
                                      bufs=4)
                        nc.vector.reciprocal(rec[:], pv[h][64:65, :])
                        nc.gpsimd.partition_broadcast(bc[:], rec[:])
                        dst = attnt1 if h < 2 else attnt2
                        r0 = 64 * (h % 2)
                        nc.vector.tensor_mul(dst[r0:r0 + 64, gq],
                                             pv[h][0:64, :], bc[:])

        # ---------------- output projection ----------------
        with tc.tile_pool(name="wop", bufs=1) as wop, \
             tc.tile_pool(name="wos", bufs=2) as wos, \
             tc.tile_pool(name="wops", bufs=2, space="PSUM") as wops:
            wo1_sb = wop.tile([KC, DIM], F32R)
            wo2_sb = wop.tile([KC, DIM], F32R)
            nc.sync.dma_start(wo1_sb[:], wo1[:])
            nc.sync.dma_start(wo2_sb[:], wo2[:])
            for m in range(TOK // KC):
                msl = slice(m * KC, (m + 1) * KC)
                o_ps = wops.tile([KC, DIM], F32, tag="o_ps")
                for n in range(DIM // CHUNK):
                    nsl = slice(n * CHUNK, (n + 1) * CHUNK)
                    nc.tensor.matmul(o_ps[:, nsl], attnt1[:, msl],
                                     wo1_sb[:, nsl], start=True, stop=False)
                    nc.tensor.matmul(o_ps[:, nsl], attnt2[:, msl],
                                     wo2_sb[:, nsl], start=False, stop=True)
                o_sb = wos.tile([KC, DIM], F32, tag="o_sb")
                nc.vector.tensor_copy(o_sb[:, 0:1024], o_ps[:, 0:1024])
                nc.scalar.copy(o_sb[:, 1024:2048], o_ps[:, 1024:2048])
                nc.sync.dma_start(out[msl, :], o_sb[:])

    nc.compile()
    return nc


def _classify(mask):
    """Classify (qc, kc) tiles. Returns (tile_types, generic)."""
    masked = mask <= -1e8
    zero = mask == 0.0
    tri = np.tril(np.ones((SEQ, SEQ), dtype=bool))  # keep where q >= k
    causal = bool(np.all(zero | masked)) and bool(
        np.array_equal(~masked, tri))
    types = [[None] * NKC for _ in range(NQB)]
    if bool(np.all(zero)):
        for qc in range(NQB):
            for kc in range(NKC):
                types[qc][kc] = 'full'
        return types, False
    if causal:
        for qc in range(NQB):
            q0, q1 = qc * CHUNK, qc * CHUNK + CHUNK - 1
            for kc in range(NKC):
                k0, k1 = kc * KC, kc * KC + KC - 1
                if q0 >= k1:
                    types[qc][kc] = 'full'
                elif q1 < k0:
                    types[qc][kc] = 'skip'
                else:
                    types[qc][kc] = 'diag'
        return types, False
    for qc in range(NQB):
        sub_q = slice(qc * CHUNK, (qc + 1) * CHUNK)
        for kc in range(NKC):
            sub = mask[sub_q, kc * KC:(kc + 1) * KC]
            if np.all(sub == 0.0):
                types[qc][kc] = 'full'
            elif np.all(sub <= -1e8):
                types[qc][kc] = 'skip'
            else:
                types[qc][kc] = 'gen'
    return types, True


def kernel(x, freqs_cos, freqs_sin, mask, wq, wk, wv, wo, cache_k, cache_v,
           start_pos):
    global LAST_RESULT
    from concourse import bass_utils

    x = np.asarray(x, dtype=np.float32)
    freqs_cos = np.asarray(freqs_cos, dtype=np.float32)
    freqs_sin = np.asarray(freqs_sin, dtype=np.float32)
    mask = np.asarray(mask, dtype=np.float32)
    wq = np.asarray(wq, dtype=np.float32)
    wk = np.asarray(wk, dtype=np.float32)
    wv = np.asarray(wv, dtype=np.float32)
    wo = np.asarray(wo, dtype=np.float32)
    assert int(start_pos) == 0, "kernel assumes start_pos == 0"

    tile_types, generic = _classify(mask)
    key = (tuple(tuple(r) for r in tile_types), generic)
    if key not in _CACHE:
        _CACHE[key] = _build(tile_types, generic)
    nc = _CACHE[key]

    xt = np.ascontiguousarray(x.reshape(TOK, DIM).T)
    cos2 = np.concatenate([freqs_cos.T, freqs_cos.T], axis=1)  # [32, 4096]
    sin2 = np.concatenate([freqs_sin.T, freqs_sin.T], axis=1)
    cos_q = np.ascontiguousarray(np.tile(cos2, (4, 1)))
    sin_q = np.ascontiguousarray(np.tile(sin2, (4, 1)))
    maskt = np.ascontiguousarray(mask.T) if generic else None

    ev = np.arange(0, HEAD_DIM, 2)
    od = np.arange(1, HEAD_DIM, 2)
    in_maps = []
    for c in range(N_CORES):
        heads = [HPC * c + i for i in range(HPC)]
        qa_cols = np.concatenate([h * HEAD_DIM + ev for h in heads])
        qb_cols = np.concatenate([h * HEAD_DIM + od for h in heads])
        wq_shard = np.ascontiguousarray(
            np.concatenate([wq[:, qa_cols], wq[:, qb_cols]], axis=1))
        wkv = np.ascontiguousarray(np.concatenate(
            [wk[:, c * HEAD_DIM + ev], wk[:, c * HEAD_DIM + od],
             wv[:, c * HEAD_DIM:(c + 1) * HEAD_DIM]], axis=1))
        wo_rows = wo[heads[0] * HEAD_DIM:(heads[-1] + 1) * HEAD_DIM, :]
        m = {"xt": xt, "cos_q": cos_q, "sin_q": sin_q,
             "wq": wq_shard, "wkv": wkv,
             "wo1": np.ascontiguousarray(wo_rows[0:128]),
             "wo2": np.ascontiguousarray(wo_rows[128:256])}
        if generic:
            m["maskt"] = maskt
        in_maps.append(m)

    res = bass_utils.run_bass_kernel_spmd(nc, in_maps, list(range(N_CORES)))
    LAST_RESULT = res
    total = np.zeros((TOK, DIM), dtype=np.float64)
    for c in range(N_CORES):
        total += res.results[c]["out"]
    return total.astype(np.float32).reshape(BATCH, SEQ, DIM)


# revision 7
# speedup vs baseline: 1.3401x; 1.3401x over previous
"""Trainium2 Bass kernel for nn_Attention_56736517980223.

Full-input contract: kernel(**inputs) takes the unsharded inputs and returns
the full [2, 2048, 2048] attention output. Internally: tensor-parallel over
heads across 8 NeuronCores (1 KV head + 4 Q heads per core); each core
computes its heads' attention and a partial x@wo contribution; the host sums
the 8 partials.

Device-side dataflow per core (all matmuls in float32r = TF32):
  - xT (host-pretransposed [dim, tok]) streams as the moving operand of the
    QKV projections; weight columns are even/odd-permuted so RoPE acts on
    contiguous 32-partition blocks.
  - scores are computed transposed (S^T[k, q] = kT.T-chunk @ qT) so softmax
    needs no reductions: exp(scale*s) runs on ScalarE straight out of PSUM,
    causal masking is a staircase zero-fill (gpsimd affine_select), and the
    row sums fall out of the PV matmul via a ones-column appended to V.
  - PV output [d+1, q] is normalized with reciprocal + partition_broadcast
    and written into attnT, which feeds the wo matmul as the stationary
    operand. Final PSUM evacuation is split across VectorE and ScalarE.
"""

import numpy as np

DIM = 2048
N_HEADS = 32
N_KV_HEADS = 8
HEAD_DIM = 64
BATCH = 2
SEQ = 2048
TOK = BATCH * SEQ  # 4096
N_CORES = 8
HPC = N_HEADS // N_KV_HEADS  # 4 q heads per core
CHUNK = 512  # token chunk (projection streaming / q block)
KC = 128     # key chunk (scores partition dim)
NQB = SEQ // CHUNK   # 4 q blocks per batch
NKC = SEQ // KC      # 16 key chunks per batch
SCALE = 1.0 / np.sqrt(HEAD_DIM)

_CACHE = {}
LAST_RESULT = None


def _build(tile_types, generic):
    """Build the SPMD Bass program.

    tile_types[qc][kc] in {'full', 'diag', 'skip', 'gen'} (batch-local,
    shared across batches and heads). 'diag' uses the causal affine_select;
    'gen' adds a DMA'd mask tile (only in generic mode).
    """
    from contextlib import ExitStack
    import concourse.bass as bass
    import concourse.tile as tile
    from concourse import bacc, mybir

    F32 = mybir.dt.float32
    F32R = mybir.dt.float32r
    BF16 = mybir.dt.bfloat16
    U16 = mybir.dt.uint16
    AF = mybir.ActivationFunctionType
    ALU = mybir.AluOpType

    nc = bacc.Bacc("TRN2", target_bir_lowering=False, debug=False,
                   num_devices=N_CORES)

    xt = nc.dram_tensor("xt", [DIM, TOK], F32R, kind="ExternalInput").ap()
    wq = nc.dram_tensor("wq", [DIM, 2 * KC], F32R, kind="ExternalInput").ap()
    wkv = nc.dram_tensor("wkv", [DIM, KC], F32R, kind="ExternalInput").ap()
    wo1 = nc.dram_tensor("wo1", [KC, DIM], F32R, kind="ExternalInput").ap()
    wo2 = nc.dram_tensor("wo2", [KC, DIM], F32R, kind="ExternalInput").ap()
    cos_q = nc.dram_tensor("cos_q", [KC, TOK], F32, kind="ExternalInput").ap()
    sin_q = nc.dram_tensor("sin_q", [KC, TOK], F32, kind="ExternalInput").ap()
    if generic:
        maskt = nc.dram_tensor("maskt", [SEQ, SEQ], F32,
                               kind="ExternalInput").ap()
    out = nc.dram_tensor("out", [TOK, DIM], F32, kind="ExternalOutput").ap()

    NCH = TOK // CHUNK  # 8 token chunks
    NKT = DIM // KC     # 16 contraction tiles for projections

    with tile.TileContext(nc) as tc, ExitStack() as ctx:
        persist = ctx.enter_context(tc.tile_pool(name="persist", bufs=1))
        qt1 = persist.tile([KC, TOK], BF16)   # heads 0,1 (rows 0:64 / 64:128)
        qt2 = persist.tile([KC, TOK], BF16)   # heads 2,3
        kt = persist.tile([KC, TOK], BF16)    # rows 0:64 = kT, 64:128 = dup
        ident = persist.tile([64, 64], BF16)
        nc.gpsimd.memset(ident[:].bitcast(U16), 0)
        nc.gpsimd.affine_select(
            out=ident[:], in_=ident[:], compare_op=ALU.not_equal,
            fill=1.0, base=0, channel_multiplier=1, pattern=[[-1, 64]])

        vpool = ctx.enter_context(tc.tile_pool(name="vpool", bufs=1))
        vt = vpool.tile([64, TOK], BF16)
        v_all = vpool.tile([KC, (TOK // KC) * 65], BF16)  # 32 [128,65] blocks

        # ---------------- projection + RoPE + V transpose ----------------
        with tc.tile_pool(name="proj", bufs=1) as proj, \
             tc.tile_pool(name="projs", bufs=2) as projs, \
             tc.tile_pool(name="ropet", bufs=2) as ropet, \
             tc.tile_pool(name="pps", bufs=2, space="PSUM") as pps:
            wq_sb = proj.tile([KC, NKT * 2 * KC], F32R)
            wkv_sb = proj.tile([KC, NKT * KC], F32R)
            nc.sync.dma_start(
                wq_sb[:].rearrange("p (t m) -> p t m", t=NKT),
                wq.rearrange("(t p) m -> p t m", p=KC))
            nc.sync.dma_start(
                wkv_sb[:].rearrange("p (t m) -> p t m", t=NKT),
                wkv.rearrange("(t p) m -> p t m", p=KC))

            for ch in range(NCH):
                tsl = slice(ch * CHUNK, (ch + 1) * CHUNK)
                xt_sb = []
                for kti in range(NKT):
                    xtile = projs.tile([KC, CHUNK], F32R, name=f"xtile{kti}",
                                       tag=f"xtile{kti}")
                    nc.sync.dma_start(
                        xtile[:], xt[kti * KC:(kti + 1) * KC, tsl])
                    xt_sb.append(xtile)
                cos_sb = projs.tile([KC, CHUNK], F32)
                sin_sb = projs.tile([KC, CHUNK], F32)
                nc.sync.dma_start(cos_sb[:], cos_q[:, tsl])
                nc.sync.dma_start(sin_sb[:], sin_q[:, tsl])

                a_ps = pps.tile([KC, CHUNK], F32, tag="a_ps")
                b_ps = pps.tile([KC, CHUNK], F32, tag="b_ps")
                kv_ps = pps.tile([KC, CHUNK], F32, tag="kv_ps")
                for kti in range(NKT):
                    st = (kti == 0)
                    sp = (kti == NKT - 1)
                    nc.tensor.matmul(
                        a_ps[:], wq_sb[:, kti * 256:kti * 256 + 128],
                        xt_sb[kti][:], start=st, stop=sp)
                    nc.tensor.matmul(
                        b_ps[:], wq_sb[:, kti * 256 + 128:kti * 256 + 256],
                        xt_sb[kti][:], start=st, stop=sp)
                    nc.tensor.matmul(
                        kv_ps[:], wkv_sb[:, kti * 128:kti * 128 + 128],
                        xt_sb[kti][:], start=st, stop=sp)

                # Q RoPE: A' = A*c - B*s ; B' = A*s + B*c  (to temps, then
                # DMA-rearrange into per-head-contiguous qt1/qt2)
                t1 = ropet.tile([KC, CHUNK], F32, tag="t1")
                t2 = ropet.tile([KC, CHUNK], F32, tag="t2")
                qa = ropet.tile([KC, CHUNK], BF16, tag="qa")
                qb = ropet.tile([KC, CHUNK], BF16, tag="qb")
                nc.vector.tensor_mul(t1[:], a_ps[:], cos_sb[:])
                nc.vector.tensor_mul(t2[:], b_ps[:], sin_sb[:])
                nc.vector.tensor_sub(qa[:], t1[:], t2[:])
                t3 = ropet.tile([KC, CHUNK], F32, tag="t1")
                t4 = ropet.tile([KC, CHUNK], F32, tag="t2")
                nc.vector.tensor_mul(t3[:], a_ps[:], sin_sb[:])
                nc.vector.tensor_mul(t4[:], b_ps[:], cos_sb[:])
                nc.vector.tensor_add(qb[:], t3[:], t4[:])
                for h in range(HPC):
                    dst = qt1 if h < 2 else qt2
                    r0 = 64 * (h % 2)
                    nc.sync.dma_start(dst[r0:r0 + 32, tsl],
                                      qa[32 * h:32 * h + 32, :])
                    nc.sync.dma_start(dst[r0 + 32:r0 + 64, tsl],
                                      qb[32 * h:32 * h + 32, :])

                # K RoPE into kt rows 0:64 (kv_ps rows 0:32=x0, 32:64=x1)
                k1 = ropet.tile([32, CHUNK], F32, tag="k1")
                k2 = ropet.tile([32, CHUNK], F32, tag="k2")
                nc.vector.tensor_mul(k1[:], kv_ps[0:32, :], cos_sb[0:32, :])
                nc.vector.tensor_mul(k2[:], kv_ps[32:64, :], sin_sb[0:32, :])
                nc.vector.tensor_sub(kt[0:32, tsl], k1[:], k2[:])
                k3 = ropet.tile([32, CHUNK], F32, tag="k1")
                k4 = ropet.tile([32, CHUNK], F32, tag="k2")
                nc.vector.tensor_mul(k3[:], kv_ps[0:32, :], sin_sb[0:32, :])
                nc.vector.tensor_mul(k4[:], kv_ps[32:64, :], cos_sb[0:32, :])
                nc.vector.tensor_add(kt[32:64, tsl], k3[:], k4[:])
                # duplicate kT rows for base-64 rhs matmuls
                nc.sync.dma_start(kt[64:128, tsl], kt[0:64, tsl])
                # V: evacuate psum rows 64:128 to vt
                nc.vector.tensor_copy(vt[0:64, tsl], kv_ps[64:128, :])

            # V transpose: vt [64, tok] -> v_all blocks [128, 65] ([V | 1])
            for blk in range(TOK // KC):
                vp = pps.tile([KC, 64], BF16, tag="vp")
                nc.tensor.transpose(
                    vp[:], vt[0:64, blk * KC:(blk + 1) * KC], ident[:])
                nc.vector.tensor_copy(v_all[:, blk * 65:blk * 65 + 64], vp[:])
                nc.gpsimd.memset(
                    v_all[:, blk * 65 + 64:blk * 65 + 65].bitcast(U16), 16256)

        # ---------------- attention ----------------
        attn = ctx.enter_context(tc.tile_pool(name="attn", bufs=1))
        attnt1 = attn.tile([KC, TOK], F32R)
        attnt2 = attn.tile([KC, TOK], F32R)

        with tc.tile_pool(name="att", bufs=2) as att, \
             tc.tile_pool(name="atps", bufs=1, space="PSUM") as atps:
            for b in range(BATCH):
                for qc in range(NQB):
                    q0 = qc * CHUNK                    # batch-local q offset
                    gq = slice(b * SEQ + q0, b * SEQ + q0 + CHUNK)
                    kcs = [k for k in range(NKC)
                           if tile_types[qc][k] != 'skip']
                    pv = [atps.tile([65, CHUNK], F32, tag=f"pv{h}",
                                    name=f"pv{h}")
                          for h in range(HPC)]
                    for i, kci in enumerate(kcs):
                        k0 = kci * KC
                        gk = slice(b * SEQ + k0, b * SEQ + k0 + KC)
                        ty = tile_types[qc][kci]
                        st = (i == 0)
                        sp = (i == len(kcs) - 1)
                        for pair in range(2):
                            qt = qt1 if pair == 0 else qt2
                            s_ps = atps.tile([KC, 2 * CHUNK], F32,
                                             tag=f"s{pair}", name=f"s{pair}")
                            nc.tensor.matmul(
                                s_ps[:, 0:CHUNK], kt[0:64, gk],
                                qt[0:64, gq], start=True, stop=True)
                            nc.tensor.matmul(
                                s_ps[:, CHUNK:], kt[64:128, gk],
                                qt[64:128, gq], start=True, stop=True)
                            ex = att.tile([KC, 2 * CHUNK], BF16,
                                          tag=f"ex{pair}", name=f"ex{pair}")
                            if ty == 'gen':
                                mt = att.tile([KC, CHUNK], F32, tag="mt",
                                              name="mt", bufs=4)
                                nc.sync.dma_start(
                                    mt[:], maskt[k0:k0 + KC, q0:q0 + CHUNK])
                                for hh in range(2):
                                    csl = slice(hh * CHUNK, (hh + 1) * CHUNK)
                                    tm = att.tile([KC, CHUNK], F32, tag="tm",
                                                  name="tm", bufs=4)
                                    nc.vector.scalar_tensor_tensor(
                                        tm[:], s_ps[:, csl], SCALE, mt[:],
                                        op0=ALU.mult, op1=ALU.add)
                                    nc.scalar.activation(
                                        ex[:, csl], tm[:], AF.Exp)
                            else:
                                nc.scalar.activation(
                                    ex[:], s_ps[:], AF.Exp, scale=SCALE)
                                if ty == 'diag':
                                    for hh in range(2):
                                        csl = slice(hh * CHUNK,
                                                    (hh + 1) * CHUNK)
                                        nc.gpsimd.affine_select(
                                            out=ex[:, csl], in_=ex[:, csl],
                                            compare_op=ALU.is_ge, fill=0.0,
                                            base=q0 - k0,
                                            channel_multiplier=-1,
                                            pattern=[[1, CHUNK]])
                            vblk = (b * SEQ + k0) // KC
                            for hh in range(2):
                                h = 2 * pair + hh
                                csl = slice(hh * CHUNK, (hh + 1) * CHUNK)
                                nc.tensor.matmul(
                                    pv[h][:],
                                    v_all[:, vblk * 65:vblk * 65 + 65],
                                    ex[:, csl], start=st, stop=sp)
                    for h in range(HPC):
                        srow = att.tile([1, CHUNK], F32, tag="srow",
                                        name="srow", bufs=4)
                        rec = att.tile([1, CHUNK], F32, tag="rec", name="rec",
                                       bufs=4)
                        bc = att.tile([64, CHUNK], F32, tag="bc", name="bc",
                                      bufs=4)
                        nc.vector.tensor_copy(srow[:], pv[h][64:65, :])
                        nc.vector.reciprocal_approx_fast(rec[:], srow[:])
                        nc.gpsimd.partition_broadcast(bc[:], rec[:])
                        dst = attnt1 if h < 2 else attnt2
                        r0 = 64 * (h % 2)
                        nc.vector.tensor_mul(dst[r0:r0 + 64, gq],
                                             pv[h][0:64, :], bc[:])

        # ---------------- output projection ----------------
        with tc.tile_pool(name="wop", bufs=1) as wop, \
             tc.tile_pool(name="wos", bufs=2) as wos, \
             tc.tile_pool(name="wops", bufs=2, space="PSUM") as wops:
            wo1_sb = wop.tile([KC, DIM], F32R)
            wo2_sb = wop.tile([KC, DIM], F32R)
            nc.sync.dma_start(wo1_sb[:], wo1[:])
            nc.sync.dma_start(wo2_sb[:], wo2[:])
            for m in range(TOK // KC):
                msl = slice(m * KC, (m + 1) * KC)
                o_ps = wops.tile([KC, DIM], F32, tag="o_ps")
                for n in range(DIM // CHUNK):
                    nsl = slice(n * CHUNK, (n + 1) * CHUNK)
                    nc.tensor.matmul(o_ps[:, nsl], attnt1[:, msl],
                                     wo1_sb[:, nsl], start=True, stop=False)
                    nc.tensor.matmul(o_ps[:, nsl], attnt2[:, msl],
                                     wo2_sb[:, nsl], start=False, stop=True)
                o_sb = wos.tile([KC, DIM], F32, tag="o_sb")
                nc.vector.tensor_copy(o_sb[:, 0:1024], o_ps[:, 0:1024])
                nc.scalar.copy(o_sb[:, 1024:2048], o_ps[:, 1024:2048])
                nc.sync.dma_start(out[msl, :], o_sb[:])

    nc.compile()
    return nc


def _classify(mask):
    """Classify (qc, kc) tiles. Returns (tile_types, generic)."""
    masked = mask <= -1e8
    zero = mask == 0.0
    tri = np.tril(np.ones((SEQ, SEQ), dtype=bool))  # keep where q >= k
    causal = bool(np.all(zero | masked)) and bool(
        np.array_equal(~masked, tri))
    types = [[None] * NKC for _ in range(NQB)]
    if bool(np.all(zero)):
        for qc in range(NQB):
            for kc in range(NKC):
                types[qc][kc] = 'full'
        return types, False
    if causal:
        for qc in range(NQB):
            q0, q1 = qc * CHUNK, qc * CHUNK + CHUNK - 1
            for kc in range(NKC):
                k0, k1 = kc * KC, kc * KC + KC - 1
                if q0 >= k1:
                    types[qc][kc] = 'full'
                elif q1 < k0:
                    types[qc][kc] = 'skip'
                else:
                    types[qc][kc] = 'diag'
        return types, False
    for qc in range(NQB):
        sub_q = slice(qc * CHUNK, (qc + 1) * CHUNK)
        for kc in range(NKC):
            sub = mask[sub_q, kc * KC:(kc + 1) * KC]
            if np.all(sub == 0.0):
                types[qc][kc] = 'full'
            elif np.all(sub <= -1e8):
                types[qc][kc] = 'skip'
            else:
                types[qc][kc] = 'gen'
    return types, True


def kernel(x, freqs_cos, freqs_sin, mask, wq, wk, wv, wo, cache_k, cache_v,
           start_pos):
    global LAST_RESULT
    from concourse import bass_utils

    x = np.asarray(x, dtype=np.float32)
    freqs_cos = np.asarray(freqs_cos, dtype=np.float32)
    freqs_sin = np.asarray(freqs_sin, dtype=np.float32)
    mask = np.asarray(mask, dtype=np.float32)
    wq = np.asarray(wq, dtype=np.float32)
    wk = np.asarray(wk, dtype=np.float32)
    wv = np.asarray(wv, dtype=np.float32)
    wo = np.asarray(wo, dtype=np.float32)
    assert int(start_pos) == 0, "kernel assumes start_pos == 0"

    tile_types, generic = _classify(mask)
    key = (tuple(tuple(r) for r in tile_types), generic)
    if key not in _CACHE:
        _CACHE[key] = _build(tile_types, generic)
    nc = _CACHE[key]

    xt = np.ascontiguousarray(x.reshape(TOK, DIM).T)
    cos2 = np.concatenate([freqs_cos.T, freqs_cos.T], axis=1)  # [32, 4096]
    sin2 = np.concatenate([freqs_sin.T, freqs_sin.T], axis=1)
    cos_q = np.ascontiguousarray(np.tile(cos2, (4, 1)))
    sin_q = np.ascontiguousarray(np.tile(sin2, (4, 1)))
    maskt = np.ascontiguousarray(mask.T) if generic else None

    ev = np.arange(0, HEAD_DIM, 2)
    od = np.arange(1, HEAD_DIM, 2)
    in_maps = []
    for c in range(N_CORES):
        heads = [HPC * c + i for i in range(HPC)]
        qa_cols = np.concatenate([h * HEAD_DIM + ev for h in heads])
        qb_cols = np.concatenate([h * HEAD_DIM + od for h in heads])
        wq_shard = np.ascontiguousarray(
            np.concatenate([wq[:, qa_cols], wq[:, qb_cols]], axis=1))
        wkv = np.ascontiguousarray(np.concatenate(
            [wk[:, c * HEAD_DIM + ev], wk[:, c * HEAD_DIM + od],
             wv[:, c * HEAD_DIM:(c + 1) * HEAD_DIM]], axis=1))
        wo_rows = wo[heads[0] * HEAD_DIM:(heads[-1] + 1) * HEAD_DIM, :]
        m = {"xt": xt, "cos_q": cos_q, "sin_q": sin_q,
             "wq": wq_shard, "wkv": wkv,
             "wo1": np.ascontiguousarray(wo_rows[0:128]),
             "wo2": np.ascontiguousarray(wo_rows[128:256])}
        if generic:
            m["maskt"] = maskt
        in_maps.append(m)

    res = bass_utils.run_bass_kernel_spmd(nc, in_maps, list(range(N_CORES)))
    LAST_RESULT = res
    total = np.zeros((TOK, DIM), dtype=np.float64)
    for c in range(N_CORES):
        total += res.results[c]["out"]
    return total.astype(np.float32).reshape(BATCH, SEQ, DIM)


# revision 10
# speedup vs baseline: 1.4673x; 1.0949x over previous
"""Trainium2 Bass kernel for nn_Attention_56736517980223.

Full-input contract: kernel(**inputs) takes the unsharded inputs and returns
the full [2, 2048, 2048] attention output. Internally: tensor-parallel over
heads across 8 NeuronCores (1 KV head + 4 Q heads per core); each core
computes its heads' attention and a partial x@wo contribution; the host sums
the 8 partials.

Device-side dataflow per core (all matmuls in float32r = TF32):
  - xT (host-pretransposed [dim, tok]) streams as the moving operand of the
    QKV projections; weight columns are even/odd-permuted so RoPE acts on
    contiguous 32-partition blocks.
  - scores are computed transposed (S^T[k, q] = kT.T-chunk @ qT) so softmax
    needs no reductions: exp(scale*s) runs on ScalarE straight out of PSUM,
    causal masking is a staircase zero-fill (gpsimd affine_select), and the
    row sums fall out of the PV matmul via a ones-column appended to V.
  - PV output [d+1, q] is normalized with reciprocal + partition_broadcast
    and written into attnT, which feeds the wo matmul as the stationary
    operand. Final PSUM evacuation is split across VectorE and ScalarE.
"""

import numpy as np

DIM = 2048
N_HEADS = 32
N_KV_HEADS = 8
HEAD_DIM = 64
BATCH = 2
SEQ = 2048
TOK = BATCH * SEQ  # 4096
N_CORES = 8
HPC = N_HEADS // N_KV_HEADS  # 4 q heads per core
CHUNK = 512  # token chunk (projection streaming / q block)
KC = 128     # key chunk (scores partition dim)
NQB = SEQ // CHUNK   # 4 q blocks per batch
NKC = SEQ // KC      # 16 key chunks per batch
SCALE = 1.0 / np.sqrt(HEAD_DIM)

_CACHE = {}
LAST_RESULT = None


def _build(tile_types, generic):
    """Build the SPMD Bass program.

    tile_types[qc][kc] in {'full', 'diag', 'skip', 'gen'} (batch-local,
    shared across batches and heads). 'diag' uses the causal affine_select;
    'gen' adds a DMA'd mask tile (only in generic mode).
    """
    from contextlib import ExitStack
    import concourse.bass as bass
    import concourse.tile as tile
    from concourse import bacc, mybir

    F32 = mybir.dt.float32
    F32R = mybir.dt.float32r
    BF16 = mybir.dt.bfloat16
    U16 = mybir.dt.uint16
    AF = mybir.ActivationFunctionType
    ALU = mybir.AluOpType

    nc = bacc.Bacc("TRN2", target_bir_lowering=False, debug=False,
                   num_devices=N_CORES)

    xt = nc.dram_tensor("xt", [DIM, TOK], BF16, kind="ExternalInput").ap()
    wq = nc.dram_tensor("wq", [DIM, 2 * KC], BF16, kind="ExternalInput").ap()
    wkv = nc.dram_tensor("wkv", [DIM, KC], BF16, kind="ExternalInput").ap()
    wo1 = nc.dram_tensor("wo1", [KC, DIM], BF16, kind="ExternalInput").ap()
    wo2 = nc.dram_tensor("wo2", [KC, DIM], BF16, kind="ExternalInput").ap()
    cos_q = nc.dram_tensor("cos_q", [KC, TOK], F32, kind="ExternalInput").ap()
    sin_q = nc.dram_tensor("sin_q", [KC, TOK], F32, kind="ExternalInput").ap()
    if generic:
        maskt = nc.dram_tensor("maskt", [SEQ, SEQ], F32,
                               kind="ExternalInput").ap()
    out = nc.dram_tensor("out", [TOK, DIM], F32, kind="ExternalOutput").ap()

    NCH = TOK // CHUNK  # 8 token chunks
    NKT = DIM // KC     # 16 contraction tiles for projections

    with tile.TileContext(nc) as tc, ExitStack() as ctx:
        persist = ctx.enter_context(tc.tile_pool(name="persist", bufs=1))
        qt1 = persist.tile([KC, TOK], BF16)   # heads 0,1 (rows 0:64 / 64:128)
        qt2 = persist.tile([KC, TOK], BF16)   # heads 2,3
        kt = persist.tile([KC, TOK], BF16)    # rows 0:64 = kT, 64:128 = dup
        ident = persist.tile([64, 64], BF16)
        nc.gpsimd.memset(ident[:].bitcast(U16), 0)
        nc.gpsimd.affine_select(
            out=ident[:], in_=ident[:], compare_op=ALU.not_equal,
            fill=1.0, base=0, channel_multiplier=1, pattern=[[-1, 64]])

        vpool = ctx.enter_context(tc.tile_pool(name="vpool", bufs=1))
        vt = vpool.tile([64, TOK], BF16)
        v_all = vpool.tile([KC, (TOK // KC) * 65], BF16)  # 32 [128,65] blocks

        # ---------------- projection + RoPE + V transpose ----------------
        with tc.tile_pool(name="proj", bufs=1) as proj, \
             tc.tile_pool(name="projs", bufs=2) as projs, \
             tc.tile_pool(name="ropet", bufs=2) as ropet, \
             tc.tile_pool(name="pps", bufs=2, space="PSUM") as pps:
            wq_sb = proj.tile([KC, NKT * 2 * KC], BF16)
            wkv_sb = proj.tile([KC, NKT * KC], BF16)
            nc.sync.dma_start(
                wq_sb[:].rearrange("p (t m) -> p t m", t=NKT),
                wq.rearrange("(t p) m -> p t m", p=KC))
            nc.sync.dma_start(
                wkv_sb[:].rearrange("p (t m) -> p t m", t=NKT),
                wkv.rearrange("(t p) m -> p t m", p=KC))

            for ch in range(NCH):
                tsl = slice(ch * CHUNK, (ch + 1) * CHUNK)
                xt_sb = []
                for kti in range(NKT):
                    xtile = projs.tile([KC, CHUNK], BF16, name=f"xtile{kti}",
                                       tag=f"xtile{kti}")
                    nc.sync.dma_start(
                        xtile[:], xt[kti * KC:(kti + 1) * KC, tsl])
                    xt_sb.append(xtile)
                cos_sb = projs.tile([KC, CHUNK], F32)
                sin_sb = projs.tile([KC, CHUNK], F32)
                nc.sync.dma_start(cos_sb[:], cos_q[:, tsl])
                nc.sync.dma_start(sin_sb[:], sin_q[:, tsl])

                a_ps = pps.tile([KC, CHUNK], F32, tag="a_ps")
                b_ps = pps.tile([KC, CHUNK], F32, tag="b_ps")
                kv_ps = pps.tile([KC, CHUNK], F32, tag="kv_ps")
                for kti in range(NKT):
                    st = (kti == 0)
                    sp = (kti == NKT - 1)
                    nc.tensor.matmul(
                        a_ps[:], wq_sb[:, kti * 256:kti * 256 + 128],
                        xt_sb[kti][:], start=st, stop=sp)
                    nc.tensor.matmul(
                        b_ps[:], wq_sb[:, kti * 256 + 128:kti * 256 + 256],
                        xt_sb[kti][:], start=st, stop=sp)
                    nc.tensor.matmul(
                        kv_ps[:], wkv_sb[:, kti * 128:kti * 128 + 128],
                        xt_sb[kti][:], start=st, stop=sp)

                # Q RoPE: A' = A*c - B*s ; B' = A*s + B*c  (to temps, then
                # DMA-rearrange into per-head-contiguous qt1/qt2)
                t1 = ropet.tile([KC, CHUNK], F32, tag="t1")
                t2 = ropet.tile([KC, CHUNK], F32, tag="t2")
                qa = ropet.tile([KC, CHUNK], BF16, tag="qa")
                qb = ropet.tile([KC, CHUNK], BF16, tag="qb")
                nc.vector.tensor_mul(t1[:], a_ps[:], cos_sb[:])
                nc.vector.tensor_mul(t2[:], b_ps[:], sin_sb[:])
                nc.vector.tensor_sub(qa[:], t1[:], t2[:])
                t3 = ropet.tile([KC, CHUNK], F32, tag="t1")
                t4 = ropet.tile([KC, CHUNK], F32, tag="t2")
                nc.vector.tensor_mul(t3[:], a_ps[:], sin_sb[:])
                nc.vector.tensor_mul(t4[:], b_ps[:], cos_sb[:])
                nc.vector.tensor_add(qb[:], t3[:], t4[:])
                for h in range(HPC):
                    dst = qt1 if h < 2 else qt2
                    r0 = 64 * (h % 2)
                    nc.sync.dma_start(dst[r0:r0 + 32, tsl],
                                      qa[32 * h:32 * h + 32, :])
                    nc.sync.dma_start(dst[r0 + 32:r0 + 64, tsl],
                                      qb[32 * h:32 * h + 32, :])

                # K RoPE into kt rows 0:64 (kv_ps rows 0:32=x0, 32:64=x1)
                k1 = ropet.tile([32, CHUNK], F32, tag="k1")
                k2 = ropet.tile([32, CHUNK], F32, tag="k2")
                nc.vector.tensor_mul(k1[:], kv_ps[0:32, :], cos_sb[0:32, :])
                nc.vector.tensor_mul(k2[:], kv_ps[32:64, :], sin_sb[0:32, :])
                nc.vector.tensor_sub(kt[0:32, tsl], k1[:], k2[:])
                k3 = ropet.tile([32, CHUNK], F32, tag="k1")
                k4 = ropet.tile([32, CHUNK], F32, tag="k2")
                nc.vector.tensor_mul(k3[:], kv_ps[0:32, :], sin_sb[0:32, :])
                nc.vector.tensor_mul(k4[:], kv_ps[32:64, :], cos_sb[0:32, :])
                nc.vector.tensor_add(kt[32:64, tsl], k3[:], k4[:])
                # duplicate kT rows for base-64 rhs matmuls
                nc.sync.dma_start(kt[64:128, tsl], kt[0:64, tsl])
                # V: evacuate psum rows 64:128 to vt
                nc.vector.tensor_copy(vt[0:64, tsl], kv_ps[64:128, :])

            # V transpose: vt [64, tok] -> v_all blocks [128, 65] ([V | 1])
            for blk in range(TOK // KC):
                vp = pps.tile([KC, 64], BF16, tag="vp")
                nc.tensor.transpose(
                    vp[:], vt[0:64, blk * KC:(blk + 1) * KC], ident[:])
                nc.vector.tensor_copy(v_all[:, blk * 65:blk * 65 + 64], vp[:])
                nc.gpsimd.memset(
                    v_all[:, blk * 65 + 64:blk * 65 + 65].bitcast(U16), 16256)

        # ---------------- attention ----------------
        attn = ctx.enter_context(tc.tile_pool(name="attn", bufs=1))
        attnt1 = attn.tile([KC, TOK], BF16)
        attnt2 = attn.tile([KC, TOK], BF16)

        with tc.tile_pool(name="att", bufs=2) as att, \
             tc.tile_pool(name="atps", bufs=1, space="PSUM") as atps:
            for b in range(BATCH):
                for qc in range(NQB):
                    q0 = qc * CHUNK                    # batch-local q offset
                    gq = slice(b * SEQ + q0, b * SEQ + q0 + CHUNK)
                    kcs = [k for k in range(NKC)
                           if tile_types[qc][k] != 'skip']
                    pv = [atps.tile([65, CHUNK], F32, tag=f"pv{h}",
                                    name=f"pv{h}")
                          for h in range(HPC)]
                    for i, kci in enumerate(kcs):
                        k0 = kci * KC
                        gk = slice(b * SEQ + k0, b * SEQ + k0 + KC)
                        ty = tile_types[qc][kci]
                        st = (i == 0)
                        sp = (i == len(kcs) - 1)
                        # diag tiles only need q >= k0: shrink to cols
                        # [w0:CHUNK) (earlier cols are fully masked)
                        w0 = max(0, k0 - q0) if ty == 'diag' else 0
                        W = CHUNK - w0
                        gqw = slice(b * SEQ + q0 + w0, b * SEQ + q0 + CHUNK)
                        for pair in range(2):
                            qt = qt1 if pair == 0 else qt2
                            s_ps = atps.tile([KC, 2 * CHUNK], F32,
                                             tag=f"s{pair}", name=f"s{pair}")
                            nc.tensor.matmul(
                                s_ps[:, w0:CHUNK], kt[0:64, gk],
                                qt[0:64, gqw], start=True, stop=True)
                            nc.tensor.matmul(
                                s_ps[:, CHUNK + w0:], kt[64:128, gk],
                                qt[64:128, gqw], start=True, stop=True)
                            ex = att.tile([KC, 2 * CHUNK], BF16,
                                          tag=f"ex{pair}", name=f"ex{pair}")
                            if ty == 'gen':
                                mt = att.tile([KC, CHUNK], F32, tag="mt",
                                              name="mt", bufs=4)
                                nc.sync.dma_start(
                                    mt[:], maskt[k0:k0 + KC, q0:q0 + CHUNK])
                                for hh in range(2):
                                    csl = slice(hh * CHUNK, (hh + 1) * CHUNK)
                                    tm = att.tile([KC, CHUNK], F32, tag="tm",
                                                  name="tm", bufs=4)
                                    nc.vector.scalar_tensor_tensor(
                                        tm[:], s_ps[:, csl], SCALE, mt[:],
                                        op0=ALU.mult, op1=ALU.add)
                                    nc.scalar.activation(
                                        ex[:, csl], tm[:], AF.Exp)
                            elif w0 == 0:
                                nc.scalar.activation(
                                    ex[:], s_ps[:], AF.Exp, scale=SCALE)
                            else:
                                for hh in range(2):
                                    csl = slice(hh * CHUNK + w0,
                                                (hh + 1) * CHUNK)
                                    nc.scalar.activation(
                                        ex[:, csl], s_ps[:, csl], AF.Exp,
                                        scale=SCALE)
                            if ty == 'diag':
                                for hh in range(2):
                                    csl = slice(hh * CHUNK + w0,
                                                (hh + 1) * CHUNK)
                                    nc.gpsimd.affine_select(
                                        out=ex[:, csl], in_=ex[:, csl],
                                        compare_op=ALU.is_ge, fill=0.0,
                                        base=q0 + w0 - k0,
                                        channel_multiplier=-1,
                                        pattern=[[1, W]])
                            vblk = (b * SEQ + k0) // KC
                            for hh in range(2):
                                h = 2 * pair + hh
                                csl = slice(hh * CHUNK + w0,
                                            (hh + 1) * CHUNK)
                                nc.tensor.matmul(
                                    pv[h][:, w0:CHUNK],
                                    v_all[:, vblk * 65:vblk * 65 + 65],
                                    ex[:, csl], start=st, stop=sp)
                    for h in range(HPC):
                        srow = att.tile([1, CHUNK], F32, tag="srow",
                                        name="srow", bufs=4)
                        rec = att.tile([1, CHUNK], F32, tag="rec", name="rec",
                                       bufs=4)
                        bc = att.tile([64, CHUNK], F32, tag="bc", name="bc",
                                      bufs=4)
                        nc.vector.tensor_copy(srow[:], pv[h][64:65, :])
                        nc.vector.reciprocal_approx_fast(rec[:], srow[:])
                        nc.gpsimd.partition_broadcast(bc[:], rec[:])
                        dst = attnt1 if h < 2 else attnt2
                        r0 = 64 * (h % 2)
                        nc.vector.tensor_mul(dst[r0:r0 + 64, gq],
                                             pv[h][0:64, :], bc[:])

        # ---------------- output projection ----------------
        with tc.tile_pool(name="wop", bufs=1) as wop, \
             tc.tile_pool(name="wos", bufs=2) as wos, \
             tc.tile_pool(name="wops", bufs=2, space="PSUM") as wops:
            wo1_sb = wop.tile([KC, DIM], BF16)
            wo2_sb = wop.tile([KC, DIM], BF16)
            nc.sync.dma_start(wo1_sb[:], wo1[:])
            nc.sync.dma_start(wo2_sb[:], wo2[:])
            for m in range(TOK // KC):
                msl = slice(m * KC, (m + 1) * KC)
                o_ps = wops.tile([KC, DIM], F32, tag="o_ps")
                for n in range(DIM // CHUNK):
                    nsl = slice(n * CHUNK, (n + 1) * CHUNK)
                    nc.tensor.matmul(o_ps[:, nsl], attnt1[:, msl],
                                     wo1_sb[:, nsl], start=True, stop=False)
                    nc.tensor.matmul(o_ps[:, nsl], attnt2[:, msl],
                                     wo2_sb[:, nsl], start=False, stop=True)
                o_sb = wos.tile([KC, DIM], F32, tag="o_sb")
                nc.vector.tensor_copy(o_sb[:, 0:1024], o_ps[:, 0:1024])
                nc.scalar.copy(o_sb[:, 1024:2048], o_ps[:, 1024:2048])
                nc.sync.dma_start(out[msl, :], o_sb[:])

    nc.compile()
    return nc


def _classify(mask):
    """Classify (qc, kc) tiles. Returns (tile_types, generic)."""
    masked = mask <= -1e8
    zero = mask == 0.0
    tri = np.tril(np.ones((SEQ, SEQ), dtype=bool))  # keep where q >= k
    causal = bool(np.all(zero | masked)) and bool(
        np.array_equal(~masked, tri))
    types = [[None] * NKC for _ in range(NQB)]
    if bool(np.all(zero)):
        for qc in range(NQB):
            for kc in range(NKC):
                types[qc][kc] = 'full'
        return types, False
    if causal:
        for qc in range(NQB):
            q0, q1 = qc * CHUNK, qc * CHUNK + CHUNK - 1
            for kc in range(NKC):
                k0, k1 = kc * KC, kc * KC + KC - 1
                if q0 >= k1:
                    types[qc][kc] = 'full'
                elif q1 < k0:
                    types[qc][kc] = 'skip'
                else:
                    types[qc][kc] = 'diag'
        return types, False
    for qc in range(NQB):
        sub_q = slice(qc * CHUNK, (qc + 1) * CHUNK)
        for kc in range(NKC):
            sub = mask[sub_q, kc * KC:(kc + 1) * KC]
            if np.all(sub == 0.0):
                types[qc][kc] = 'full'
            elif np.all(sub <= -1e8):
                types[qc][kc] = 'skip'
            else:
                types[qc][kc] = 'gen'
    return types, True


def kernel(x, freqs_cos, freqs_sin, mask, wq, wk, wv, wo, cache_k, cache_v,
           start_pos):
    global LAST_RESULT
    from concourse import bass_utils

    x = np.asarray(x, dtype=np.float32)
    freqs_cos = np.asarray(freqs_cos, dtype=np.float32)
    freqs_sin = np.asarray(freqs_sin, dtype=np.float32)
    mask = np.asarray(mask, dtype=np.float32)
    wq = np.asarray(wq, dtype=np.float32)
    wk = np.asarray(wk, dtype=np.float32)
    wv = np.asarray(wv, dtype=np.float32)
    wo = np.asarray(wo, dtype=np.float32)
    assert int(start_pos) == 0, "kernel assumes start_pos == 0"

    tile_types, generic = _classify(mask)
    key = (tuple(tuple(r) for r in tile_types), generic)
    if key not in _CACHE:
        _CACHE[key] = _build(tile_types, generic)
    nc = _CACHE[key]

    import ml_dtypes
    bf16 = ml_dtypes.bfloat16
    xt = np.ascontiguousarray(x.reshape(TOK, DIM).T).astype(bf16)
    cos2 = np.concatenate([freqs_cos.T, freqs_cos.T], axis=1)  # [32, 4096]
    sin2 = np.concatenate([freqs_sin.T, freqs_sin.T], axis=1)
    cos_q = np.ascontiguousarray(np.tile(cos2, (4, 1)))
    sin_q = np.ascontiguousarray(np.tile(sin2, (4, 1)))
    maskt = np.ascontiguousarray(mask.T) if generic else None

    ev = np.arange(0, HEAD_DIM, 2)
    od = np.arange(1, HEAD_DIM, 2)
    in_maps = []
    for c in range(N_CORES):
        heads = [HPC * c + i for i in range(HPC)]
        qa_cols = np.concatenate([h * HEAD_DIM + ev for h in heads])
        qb_cols = np.concatenate([h * HEAD_DIM + od for h in heads])
        wq_shard = np.ascontiguousarray(
            np.concatenate([wq[:, qa_cols], wq[:, qb_cols]],
                           axis=1)).astype(bf16)
        wkv = np.ascontiguousarray(np.concatenate(
            [wk[:, c * HEAD_DIM + ev], wk[:, c * HEAD_DIM + od],
             wv[:, c * HEAD_DIM:(c + 1) * HEAD_DIM]], axis=1)).astype(bf16)
        wo_rows = wo[heads[0] * HEAD_DIM:(heads[-1] + 1) * HEAD_DIM, :]
        m = {"xt": xt, "cos_q": cos_q, "sin_q": sin_q,
             "wq": wq_shard, "wkv": wkv,
             "wo1": np.ascontiguousarray(wo_rows[0:128]).astype(bf16),
             "wo2": np.ascontiguousarray(wo_rows[128:256]).astype(bf16)}
        if generic:
            m["maskt"] = maskt
        in_maps.append(m)

    res = bass_utils.run_bass_kernel_spmd(nc, in_maps, list(range(N_CORES)))
    LAST_RESULT = res
    total = np.zeros((TOK, DIM), dtype=np.float64)
    for c in range(N_CORES):
        total += res.results[c]["out"]
    return total.astype(np.float32).reshape(BATCH, SEQ, DIM)


# revision 12
# speedup vs baseline: 1.6086x; 1.0963x over previous
"""Trainium2 Bass kernel for nn_Attention_56736517980223.

Full-input contract: kernel(**inputs) takes the unsharded inputs and returns
the full [2, 2048, 2048] attention output. Internally: tensor-parallel over
heads across 8 NeuronCores (1 KV head + 4 Q heads per core); each core
computes its heads' attention and a partial x@wo contribution; the host sums
the 8 partials.

Device-side dataflow per core (all matmuls in float32r = TF32):
  - xT (host-pretransposed [dim, tok]) streams as the moving operand of the
    QKV projections; weight columns are even/odd-permuted so RoPE acts on
    contiguous 32-partition blocks.
  - scores are computed transposed (S^T[k, q] = kT.T-chunk @ qT) so softmax
    needs no reductions: exp(scale*s) runs on ScalarE straight out of PSUM,
    causal masking is a staircase zero-fill (gpsimd affine_select), and the
    row sums fall out of the PV matmul via a ones-column appended to V.
  - PV output [d+1, q] is normalized with reciprocal + partition_broadcast
    and written into attnT, which feeds the wo matmul as the stationary
    operand. Final PSUM evacuation is split across VectorE and ScalarE.
"""

import numpy as np

DIM = 2048
N_HEADS = 32
N_KV_HEADS = 8
HEAD_DIM = 64
BATCH = 2
SEQ = 2048
TOK = BATCH * SEQ  # 4096
N_CORES = 8
HPC = N_HEADS // N_KV_HEADS  # 4 q heads per core
CHUNK = 512  # token chunk (projection streaming / q block)
KC = 128     # key chunk (scores partition dim)
NQB = SEQ // CHUNK   # 4 q blocks per batch
NKC = SEQ // KC      # 16 key chunks per batch
SCALE = 1.0 / np.sqrt(HEAD_DIM)

_CACHE = {}
LAST_RESULT = None


def _build(tile_types, generic):
    """Build the SPMD Bass program.

    tile_types[qc][kc] in {'full', 'diag', 'skip', 'gen'} (batch-local,
    shared across batches and heads). 'diag' uses the causal affine_select;
    'gen' adds a DMA'd mask tile (only in generic mode).
    """
    from contextlib import ExitStack
    import concourse.bass as bass
    import concourse.tile as tile
    from concourse import bacc, mybir

    F32 = mybir.dt.float32
    F32R = mybir.dt.float32r
    BF16 = mybir.dt.bfloat16
    U16 = mybir.dt.uint16
    AF = mybir.ActivationFunctionType
    ALU = mybir.AluOpType

    nc = bacc.Bacc("TRN2", target_bir_lowering=False, debug=False,
                   num_devices=N_CORES)

    xt = nc.dram_tensor("xt", [DIM, TOK], BF16, kind="ExternalInput").ap()
    wq = nc.dram_tensor("wq", [DIM, 2 * KC], BF16, kind="ExternalInput").ap()
    wkv = nc.dram_tensor("wkv", [DIM, KC], BF16, kind="ExternalInput").ap()
    wo1 = nc.dram_tensor("wo1", [KC, DIM], BF16, kind="ExternalInput").ap()
    wo2 = nc.dram_tensor("wo2", [KC, DIM], BF16, kind="ExternalInput").ap()
    cos_q = nc.dram_tensor("cos_q", [KC, TOK], F32, kind="ExternalInput").ap()
    sin_q = nc.dram_tensor("sin_q", [KC, TOK], F32, kind="ExternalInput").ap()
    if generic:
        maskt = nc.dram_tensor("maskt", [SEQ, SEQ], F32,
                               kind="ExternalInput").ap()
    out = nc.dram_tensor("out", [TOK, DIM], F32, kind="ExternalOutput").ap()

    NCH = TOK // CHUNK  # 8 token chunks
    NKT = DIM // KC     # 16 contraction tiles for projections

    with tile.TileContext(nc) as tc, ExitStack() as ctx:
        persist = ctx.enter_context(tc.tile_pool(name="persist", bufs=1))
        qt1 = persist.tile([KC, TOK], BF16)   # heads 0,1 (rows 0:64 / 64:128)
        qt2 = persist.tile([KC, TOK], BF16)   # heads 2,3
        kt = persist.tile([KC, TOK], BF16)    # rows 0:64 = kT, 64:128 = dup
        ident = persist.tile([64, 64], BF16)
        nc.gpsimd.memset(ident[:].bitcast(U16), 0)
        nc.gpsimd.affine_select(
            out=ident[:], in_=ident[:], compare_op=ALU.not_equal,
            fill=1.0, base=0, channel_multiplier=1, pattern=[[-1, 64]])

        vpool = ctx.enter_context(tc.tile_pool(name="vpool", bufs=1))
        vt = vpool.tile([64, TOK], BF16)
        v_all = vpool.tile([KC, (TOK // KC) * 65], BF16)  # 32 [128,65] blocks

        # ---------------- projection + RoPE + V transpose ----------------
        with tc.tile_pool(name="proj", bufs=1) as proj, \
             tc.tile_pool(name="projs", bufs=2) as projs, \
             tc.tile_pool(name="ropet", bufs=2) as ropet, \
             tc.tile_pool(name="pps", bufs=2, space="PSUM") as pps:
            wq_sb = proj.tile([KC, NKT * 2 * KC], BF16)
            wkv_sb = proj.tile([KC, NKT * KC], BF16)
            nc.sync.dma_start(
                wq_sb[:].rearrange("p (t m) -> p t m", t=NKT),
                wq.rearrange("(t p) m -> p t m", p=KC))
            nc.sync.dma_start(
                wkv_sb[:].rearrange("p (t m) -> p t m", t=NKT),
                wkv.rearrange("(t p) m -> p t m", p=KC))

            for ch in range(NCH):
                tsl = slice(ch * CHUNK, (ch + 1) * CHUNK)
                # whole x chunk [dim, CHUNK] in one tile / one DMA: the PE
                # stream takes a single wait instead of 16
                xc = projs.tile([KC, NKT * CHUNK], BF16, tag="xc")
                nc.sync.dma_start(
                    xc[:].rearrange("p (t n) -> p t n", t=NKT),
                    xt[:, tsl].rearrange("(t p) n -> p t n", p=KC))
                cos_sb = projs.tile([KC, CHUNK], F32)
                sin_sb = projs.tile([KC, CHUNK], F32)
                nc.sync.dma_start(cos_sb[:], cos_q[:, tsl])
                nc.sync.dma_start(sin_sb[:], sin_q[:, tsl])

                a_ps = pps.tile([KC, CHUNK], F32, tag="a_ps")
                b_ps = pps.tile([KC, CHUNK], F32, tag="b_ps")
                kv_ps = pps.tile([KC, CHUNK], F32, tag="kv_ps")
                # group matmuls per accumulator: dense same-bank runs keep
                # the PE stream from cycling PSUM banks every instruction
                for ps_t, woff, wsb in ((a_ps, 0, wq_sb), (b_ps, 128, wq_sb),
                                        (kv_ps, 0, wkv_sb)):
                    stride = 256 if wsb is wq_sb else 128
                    for kti in range(NKT):
                        nc.tensor.matmul(
                            ps_t[:],
                            wsb[:, kti * stride + woff:
                                kti * stride + woff + 128],
                            xc[:, kti * CHUNK:(kti + 1) * CHUNK],
                            start=(kti == 0), stop=(kti == NKT - 1))

                # Q RoPE: A' = A*c - B*s ; B' = A*s + B*c  (to temps, then
                # DMA-rearrange into per-head-contiguous qt1/qt2)
                t1 = ropet.tile([KC, CHUNK], F32, tag="t1")
                t2 = ropet.tile([KC, CHUNK], F32, tag="t2")
                qa = ropet.tile([KC, CHUNK], BF16, tag="qa")
                qb = ropet.tile([KC, CHUNK], BF16, tag="qb")
                nc.vector.tensor_mul(t1[:], a_ps[:], cos_sb[:])
                nc.vector.tensor_mul(t2[:], b_ps[:], sin_sb[:])
                nc.vector.tensor_sub(qa[:], t1[:], t2[:])
                t3 = ropet.tile([KC, CHUNK], F32, tag="t1")
                t4 = ropet.tile([KC, CHUNK], F32, tag="t2")
                nc.vector.tensor_mul(t3[:], a_ps[:], sin_sb[:])
                nc.vector.tensor_mul(t4[:], b_ps[:], cos_sb[:])
                nc.vector.tensor_add(qb[:], t3[:], t4[:])
                for h in range(HPC):
                    dst = qt1 if h < 2 else qt2
                    r0 = 64 * (h % 2)
                    nc.sync.dma_start(dst[r0:r0 + 32, tsl],
                                      qa[32 * h:32 * h + 32, :])
                    nc.sync.dma_start(dst[r0 + 32:r0 + 64, tsl],
                                      qb[32 * h:32 * h + 32, :])

                # K RoPE into kt rows 0:64 (kv_ps rows 0:32=x0, 32:64=x1)
                k1 = ropet.tile([32, CHUNK], F32, tag="k1")
                k2 = ropet.tile([32, CHUNK], F32, tag="k2")
                nc.vector.tensor_mul(k1[:], kv_ps[0:32, :], cos_sb[0:32, :])
                nc.vector.tensor_mul(k2[:], kv_ps[32:64, :], sin_sb[0:32, :])
                nc.vector.tensor_sub(kt[0:32, tsl], k1[:], k2[:])
                k3 = ropet.tile([32, CHUNK], F32, tag="k1")
                k4 = ropet.tile([32, CHUNK], F32, tag="k2")
                nc.vector.tensor_mul(k3[:], kv_ps[0:32, :], sin_sb[0:32, :])
                nc.vector.tensor_mul(k4[:], kv_ps[32:64, :], cos_sb[0:32, :])
                nc.vector.tensor_add(kt[32:64, tsl], k3[:], k4[:])
                # duplicate kT rows for base-64 rhs matmuls
                nc.sync.dma_start(kt[64:128, tsl], kt[0:64, tsl])
                # V: evacuate psum rows 64:128 to vt
                nc.vector.tensor_copy(vt[0:64, tsl], kv_ps[64:128, :])

            # V transpose: vt [64, tok] -> v_all blocks [128, 65] ([V | 1])
            for blk in range(TOK // KC):
                vp = pps.tile([KC, 64], BF16, tag="vp")
                nc.tensor.transpose(
                    vp[:], vt[0:64, blk * KC:(blk + 1) * KC], ident[:])
                nc.vector.tensor_copy(v_all[:, blk * 65:blk * 65 + 64], vp[:])
                nc.gpsimd.memset(
                    v_all[:, blk * 65 + 64:blk * 65 + 65].bitcast(U16), 16256)

        # ---------------- attention ----------------
        attn = ctx.enter_context(tc.tile_pool(name="attn", bufs=1))
        attnt1 = attn.tile([KC, TOK], BF16)
        attnt2 = attn.tile([KC, TOK], BF16)

        with tc.tile_pool(name="att", bufs=2) as att, \
             tc.tile_pool(name="atps", bufs=1, space="PSUM") as atps:
            for b in range(BATCH):
                for qc in range(NQB):
                    q0 = qc * CHUNK                    # batch-local q offset
                    gq = slice(b * SEQ + q0, b * SEQ + q0 + CHUNK)
                    kcs = [k for k in range(NKC)
                           if tile_types[qc][k] != 'skip']
                    # two heads per pass: scores double-buffered (4 banks) +
                    # pv double-buffered (4 banks) so the PE streams ahead
                    # of ScalarE's exp
                    for pair in range(2):
                        qt = qt1 if pair == 0 else qt2
                        pv = [atps.tile([65, CHUNK], F32, tag=f"pv{hh}",
                                        name=f"pv{hh}", bufs=2)
                              for hh in range(2)]
                        for i, kci in enumerate(kcs):
                            k0 = kci * KC
                            gk = slice(b * SEQ + k0, b * SEQ + k0 + KC)
                            ty = tile_types[qc][kci]
                            st = (i == 0)
                            sp = (i == len(kcs) - 1)
                            # diag tiles only need q >= k0: shrink to cols
                            # [w0:CHUNK) (earlier cols are fully masked)
                            w0 = max(0, k0 - q0) if ty == 'diag' else 0
                            W = CHUNK - w0
                            gqw = slice(b * SEQ + q0 + w0,
                                        b * SEQ + q0 + CHUNK)
                            s_ps = atps.tile([KC, 2 * CHUNK], F32,
                                             tag="s", name="s_ps", bufs=2)
                            nc.tensor.matmul(
                                s_ps[:, w0:CHUNK], kt[0:64, gk],
                                qt[0:64, gqw], start=True, stop=True)
                            nc.tensor.matmul(
                                s_ps[:, CHUNK + w0:], kt[64:128, gk],
                                qt[64:128, gqw], start=True, stop=True)
                            ex = att.tile([KC, 2 * CHUNK], BF16,
                                          tag="ex", name="ex", bufs=3)
                            if ty == 'gen':
                                mt = att.tile([KC, CHUNK], F32, tag="mt",
                                              name="mt", bufs=4)
                                nc.sync.dma_start(
                                    mt[:], maskt[k0:k0 + KC, q0:q0 + CHUNK])
                                for hh in range(2):
                                    csl = slice(hh * CHUNK, (hh + 1) * CHUNK)
                                    tm = att.tile([KC, CHUNK], F32, tag="tm",
                                                  name="tm", bufs=4)
                                    nc.vector.scalar_tensor_tensor(
                                        tm[:], s_ps[:, csl], SCALE, mt[:],
                                        op0=ALU.mult, op1=ALU.add)
                                    nc.scalar.activation(
                                        ex[:, csl], tm[:], AF.Exp)
                            elif w0 == 0:
                                nc.scalar.activation(
                                    ex[:], s_ps[:], AF.Exp, scale=SCALE)
                            else:
                                for hh in range(2):
                                    csl = slice(hh * CHUNK + w0,
                                                (hh + 1) * CHUNK)
                                    nc.scalar.activation(
                                        ex[:, csl], s_ps[:, csl], AF.Exp,
                                        scale=SCALE)
                            if ty == 'diag':
                                for hh in range(2):
                                    csl = slice(hh * CHUNK + w0,
                                                (hh + 1) * CHUNK)
                                    nc.gpsimd.affine_select(
                                        out=ex[:, csl], in_=ex[:, csl],
                                        compare_op=ALU.is_ge, fill=0.0,
                                        base=q0 + w0 - k0,
                                        channel_multiplier=-1,
                                        pattern=[[1, W]])
                            vblk = (b * SEQ + k0) // KC
                            for hh in range(2):
                                csl = slice(hh * CHUNK + w0,
                                            (hh + 1) * CHUNK)
                                nc.tensor.matmul(
                                    pv[hh][:, w0:CHUNK],
                                    v_all[:, vblk * 65:vblk * 65 + 65],
                                    ex[:, csl], start=st, stop=sp)
                        for hh in range(2):
                            h = 2 * pair + hh
                            srow = att.tile([1, CHUNK], F32, tag="srow",
                                            name="srow", bufs=4)
                            rec = att.tile([1, CHUNK], F32, tag="rec",
                                           name="rec", bufs=4)
                            bc = att.tile([64, CHUNK], F32, tag="bc",
                                          name="bc", bufs=4)
                            nc.vector.tensor_copy(srow[:], pv[hh][64:65, :])
                            nc.vector.reciprocal_approx_fast(rec[:], srow[:])
                            nc.gpsimd.partition_broadcast(bc[:], rec[:])
                            dst = attnt1 if h < 2 else attnt2
                            r0 = 64 * (h % 2)
                            nc.vector.tensor_mul(dst[r0:r0 + 64, gq],
                                                 pv[hh][0:64, :], bc[:])

        # ---------------- output projection ----------------
        with tc.tile_pool(name="wop", bufs=1) as wop, \
             tc.tile_pool(name="wos", bufs=2) as wos, \
             tc.tile_pool(name="wops", bufs=2, space="PSUM") as wops:
            wo1_sb = wop.tile([KC, DIM], BF16)
            wo2_sb = wop.tile([KC, DIM], BF16)
            nc.sync.dma_start(wo1_sb[:], wo1[:])
            nc.sync.dma_start(wo2_sb[:], wo2[:])
            for m in range(TOK // KC):
                msl = slice(m * KC, (m + 1) * KC)
                o_ps = wops.tile([KC, DIM], F32, tag="o_ps")
                for n in range(DIM // CHUNK):
                    nsl = slice(n * CHUNK, (n + 1) * CHUNK)
                    nc.tensor.matmul(o_ps[:, nsl], attnt1[:, msl],
                                     wo1_sb[:, nsl], start=True, stop=False)
                    nc.tensor.matmul(o_ps[:, nsl], attnt2[:, msl],
                                     wo2_sb[:, nsl], start=False, stop=True)
                o_sb = wos.tile([KC, DIM], F32, tag="o_sb")
                nc.vector.tensor_copy(o_sb[:, 0:1024], o_ps[:, 0:1024])
                nc.scalar.copy(o_sb[:, 1024:2048], o_ps[:, 1024:2048])
                nc.sync.dma_start(out[msl, :], o_sb[:])

    nc.compile()
    return nc


def _classify(mask):
    """Classify (qc, kc) tiles. Returns (tile_types, generic)."""
    masked = mask <= -1e8
    zero = mask == 0.0
    tri = np.tril(np.ones((SEQ, SEQ), dtype=bool))  # keep where q >= k
    causal = bool(np.all(zero | masked)) and bool(
        np.array_equal(~masked, tri))
    types = [[None] * NKC for _ in range(NQB)]
    if bool(np.all(zero)):
        for qc in range(NQB):
            for kc in range(NKC):
                types[qc][kc] = 'full'
        return types, False
    if causal:
        for qc in range(NQB):
            q0, q1 = qc * CHUNK, qc * CHUNK + CHUNK - 1
            for kc in range(NKC):
                k0, k1 = kc * KC, kc * KC + KC - 1
                if q0 >= k1:
                    types[qc][kc] = 'full'
                elif q1 < k0:
                    types[qc][kc] = 'skip'
                else:
                    types[qc][kc] = 'diag'
        return types, False
    for qc in range(NQB):
        sub_q = slice(qc * CHUNK, (qc + 1) * CHUNK)
        for kc in range(NKC):
            sub = mask[sub_q, kc * KC:(kc + 1) * KC]
            if np.all(sub == 0.0):
                types[qc][kc] = 'full'
            elif np.all(sub <= -1e8):
                types[qc][kc] = 'skip'
            else:
                types[qc][kc] = 'gen'
    return types, True


def kernel(x, freqs_cos, freqs_sin, mask, wq, wk, wv, wo, cache_k, cache_v,
           start_pos):
    global LAST_RESULT
    from concourse import bass_utils

    x = np.asarray(x, dtype=np.float32)
    freqs_cos = np.asarray(freqs_cos, dtype=np.float32)
    freqs_sin = np.asarray(freqs_sin, dtype=np.float32)
    mask = np.asarray(mask, dtype=np.float32)
    wq = np.asarray(wq, dtype=np.float32)
    wk = np.asarray(wk, dtype=np.float32)
    wv = np.asarray(wv, dtype=np.float32)
    wo = np.asarray(wo, dtype=np.float32)
    assert int(start_pos) == 0, "kernel assumes start_pos == 0"

    tile_types, generic = _classify(mask)
    key = (tuple(tuple(r) for r in tile_types), generic)
    if key not in _CACHE:
        _CACHE[key] = _build(tile_types, generic)
    nc = _CACHE[key]

    import ml_dtypes
    bf16 = ml_dtypes.bfloat16
    xt = np.ascontiguousarray(x.reshape(TOK, DIM).T).astype(bf16)
    cos2 = np.concatenate([freqs_cos.T, freqs_cos.T], axis=1)  # [32, 4096]
    sin2 = np.concatenate([freqs_sin.T, freqs_sin.T], axis=1)
    cos_q = np.ascontiguousarray(np.tile(cos2, (4, 1)))
    sin_q = np.ascontiguousarray(np.tile(sin2, (4, 1)))
    maskt = np.ascontiguousarray(mask.T) if generic else None

    ev = np.arange(0, HEAD_DIM, 2)
    od = np.arange(1, HEAD_DIM, 2)
    in_maps = []
    for c in range(N_CORES):
        heads = [HPC * c + i for i in range(HPC)]
        qa_cols = np.concatenate([h * HEAD_DIM + ev for h in heads])
        qb_cols = np.concatenate([h * HEAD_DIM + od for h in heads])
        wq_shard = np.ascontiguousarray(
            np.concatenate([wq[:, qa_cols], wq[:, qb_cols]],
                           axis=1)).astype(bf16)
        wkv = np.ascontiguousarray(np.concatenate(
            [wk[:, c * HEAD_DIM + ev], wk[:, c * HEAD_DIM + od],
             wv[:, c * HEAD_DIM:(c + 1) * HEAD_DIM]], axis=1)).astype(bf16)
        wo_rows = wo[heads[0] * HEAD_DIM:(heads[-1] + 1) * HEAD_DIM, :]
        m = {"xt": xt, "cos_q": cos_q, "sin_q": sin_q,
             "wq": wq_shard, "wkv": wkv,
             "wo1": np.ascontiguousarray(wo_rows[0:128]).astype(bf16),
             "wo2": np.ascontiguousarray(wo_rows[128:256]).astype(bf16)}
        if generic:
            m["maskt"] = maskt
        in_maps.append(m)

    res = bass_utils.run_bass_kernel_spmd(nc, in_maps, list(range(N_CORES)))
    LAST_RESULT = res
    total = np.zeros((TOK, DIM), dtype=np.float64)
    for c in range(N_CORES):
        total += res.results[c]["out"]
    return total.astype(np.float32).reshape(BATCH, SEQ, DIM)
